# revision 60
# baseline (speedup 1.0000x reference)
"""Trainium2 Bass kernel for nn_BottomUpNet (dense_mlp).

Reference computation (per row n of N=8192, fully independent across rows):
    summary = aggregate (broadcast)                   # (1024,)
    for k in 0..15:
        x = [summary, towers[n, k, :]]                # (1088,)
        h = relu(x @ OW1 + Ob1); h = relu(h @ OW2 + Ob2)
        pred_k = sigmoid(h @ OW3 + Ob3)
        m = relu(x @ MW1 + Mb1); m = relu(m @ MW2 + Mb2); m = relu(m @ MW3 + Mb3)
        summary = m
    out[n] = prod_k pred_k

Strategy: data-parallel over N across 8 cores (1024 rows each), weights
replicated.  Activations are feature-major ([feature partition, row free])
so weight matrices serve directly as the stationary matmul operand and no
on-chip transposes are needed.

All matmuls run in fp8-e4m3 with perf_mode=DoubleRow: both operands carry
contraction pairs [K, 2, free] so each matmul instruction reduces 2*K rows
(2 fp8 weights per PE cell), ~1.7x the bf16 streaming rate.  The five
1024-contraction layers per step (M1s, M2, M3, O1s, O2) use [128, 2, *]
operands; weights are pre-interleaved on the host into [ktile, 128, 2,
NH]; activations feeding these matmuls are written by the epilogues as
fp8 pair-tiles [128, 2, 512].  The 64-wide tower matmuls are [32, 2, *]
DoubleRow closers of the layer-1 accumulation groups.  End-to-end rel err
vs the f32 reference ~1.1e-2 (fp8 quantization noise; the e4m3 denormal
range covers the small uniform weights acceptably, so no weight scaling
is needed and bias+relu epilogues keep their single-op form).  f32 PSUM
accumulation throughout; the output head stays bf16/f32.

Perf structure (measured 1.172 ms vs 2.249 ms for the best bf16 version;
PE busy ~96%, pinned on the DoubleRow streaming rate of ~216ns per
256x128x512 matmul):
  - loop order is m-outer / row-block-inner so each DoubleRow stationary
    tile (256x128 weight block) is reused by NR=2 matmuls, halving
    LDWEIGHTS traffic (DoubleRow weight loads are 2x the columns).
  - the four per-m layer-1 tower closers (M/O branch x 2 row blocks) are
    placed in the four disjoint 32-row PE row groups (tower data + weights
    duplicated across partition quarters), so all four stream concurrently
    in the systolic array (~216ns for the wave).
  - step 0's summary is the broadcast aggregate, identical for all rows:
    its layer-1 contribution agg @ W1s is folded into the step-0 bias on
    the host, so step 0's layer 1 is just the tower matmuls.
  - the 1024->1 output head is a DVE per-partition multiply/add tree
    (g = sum_i h2_i * w3_i) plus a single ones-vector matmul for the
    cross-partition reduce; the chain's DVE ops are emitted as deferred
    closures drained ~4 per m-iteration of the NEXT step's layer-1 loop,
    so they interleave with the epilogues in the DVE FIFO instead of
    forming a ~9us burst that delays PSUM bank release (which showed up
    as ~150ns stalls on group-opening matmuls).  The sigmoid + product-
    accumulate are deferred likewise.
  - the final step's M branch (M1/M2/M3) is skipped entirely: the
    reference discards the last scan carry, so that summary is dead; the
    final step's head runs as 8 accumulating [128,1]-stationary bf16
    matmuls per row block on the otherwise-idle PE (emitted after both
    row blocks' DoubleRow streams), so the post-matmul tail is ~5us
    instead of a ~12us serialized DVE mult/add-tree chain.
  - relu epilogues alternate between the scalar and vector engines,
    staggered so each m-iteration hits both engines once.
  - both [1, 512] head reduces share one PSUM bank at partitions 0/32
    (explicit tile_position), freeing the 8th bank for the matmul pool.
  - weight DMAs round-robin over the three DGE queues in first-use order;
    step 0 needs only the tower weights + biases + step-0 towers, which
    are issued first so the PE starts at ~10us.

Known residual costs (~90us over the ~1.09ms matmul-stream floor):
LDWEIGHTS exposure when switching between 32-row and 128-row stationaries
around each tower wave (~280ns per m-group; a full-row LDWEIGHTS cannot
be pulled ahead past in-flight partial-row matmuls and vice versa),
~7us of weight-DMA gating during steps 0-1 (5MB of fp8 weights over
three ~45GB/s queues), and chip-state variance: under sustained load the
chip drops PE 2.4->2.0GHz (P0), inflating wall time ~20% run-to-run.
"""

import numpy as np
import ml_dtypes

import concourse.bacc as bacc
import concourse.mybir as mybir
import concourse.tile as tile
from concourse.bass import ts, ds
from concourse.bass_utils import run_bass_kernel_spmd

BF16 = ml_dtypes.bfloat16
F8 = ml_dtypes.float8_e4m3

N_CORES = 8
N = 8192
K = 16
NI = 64          # tower features per step
NH = 1024        # hidden width
FT = NH // 128   # feature tiles (8)
KT = NH // 256   # DoubleRow contraction tiles (4)
R = N // N_CORES  # rows per core (1024)
RB = 512         # row block (matmul moving dim / one PSUM bank)
NR = R // RB     # row blocks per core (2)

_BUILT = None


def _build():
    nc = bacc.Bacc("TRN2", target_bir_lowering=False, debug=False,
                   num_devices=N_CORES)
    f32 = mybir.dt.float32
    bf = mybir.dt.bfloat16
    f8 = mybir.dt.float8e4
    DR = mybir.MatmulPerfMode.DoubleRow

    towd = nc.declare_dram_parameter("tow", [K, NI // 2, 2, R], f8,
                                     isOutput=False)
    mw1sd = nc.declare_dram_parameter("mw1s", [KT, 128, 2, NH], f8,
                                      isOutput=False)
    mw1td = nc.declare_dram_parameter("mw1t", [NI // 2, 2, NH], f8,
                                      isOutput=False)
    mw2d = nc.declare_dram_parameter("mw2", [KT, 128, 2, NH], f8,
                                     isOutput=False)
    mw3d = nc.declare_dram_parameter("mw3", [KT, 128, 2, NH], f8,
                                     isOutput=False)
    ow1sd = nc.declare_dram_parameter("ow1s", [KT, 128, 2, NH], f8,
                                      isOutput=False)
    ow1td = nc.declare_dram_parameter("ow1t", [NI // 2, 2, NH], f8,
                                      isOutput=False)
    ow2d = nc.declare_dram_parameter("ow2", [KT, 128, 2, NH], f8,
                                     isOutput=False)
    w3cd = nc.declare_dram_parameter("w3c", [128, FT], f32, isOutput=False)
    balld = nc.declare_dram_parameter("ball", [128, 56], f32, isOutput=False)
    ob3d = nc.declare_dram_parameter("ob3", [1, 1], f32, isOutput=False)
    outd = nc.declare_dram_parameter("out", [1, R], f32, isOutput=True)

    Relu = mybir.ActivationFunctionType.Relu
    Sigmoid = mybir.ActivationFunctionType.Sigmoid
    Identity = mybir.ActivationFunctionType.Identity
    Add = mybir.AluOpType.add
    Mult = mybir.AluOpType.mult

    with tile.TileContext(nc) as tc:
        with (
            tc.tile_pool(name="weights", bufs=1) as wp,
            tc.tile_pool(name="summary", bufs=1) as sp,
            tc.tile_pool(name="acts", bufs=16) as ap,
            tc.tile_pool(name="tow", bufs=4) as twp,
            tc.tile_pool(name="small", bufs=1) as smp,
            tc.tile_pool(name="zwork", bufs=2) as zw,
            tc.tile_pool(name="psum", bufs=7, space="PSUM") as pp,
            tc.tile_pool(name="zpsum", bufs=1, space="PSUM") as zp,
        ):
            # --- small/early tiles on the gpsimd SW queue; step 0 only
            # needs the tower weights + biases to start.  The step-0 tower
            # weights + tower data go first on the two HW DGE queues, and
            # the weights are spread over four queues in deadline order
            # (step 0: M2, M3, O2; step 1 adds M1s, O1s on vector/tensor
    # queues so they load in parallel with the step-0 weights). ---
            ball = smp.tile([128, 56], f32, tag="ball", name="ball")
            nc.gpsimd.dma_start(out=ball, in_=balld[:])
            ob3 = smp.tile([1, 1], f32, tag="ob3", name="ob3")
            nc.gpsimd.dma_start(out=ob3, in_=ob3d[:])
            w3c = smp.tile([128, FT], f32, tag="w3c", name="w3c")
            nc.gpsimd.dma_start(out=w3c, in_=w3cd[:])
            # tower weights as DoubleRow pairs, duplicated across all four
            # 32-row PE row groups (M r0 / O r0 / M r1 / O r1) so the four
            # per-m tower matmuls all stream concurrently
            w1t5 = wp.tile([128, 2, NH], f8, tag="w1t5", name="w1t5")
            tow0 = twp.tile([128, 2, R], f8, tag="tow", name="tow")

            _q = [0]
            _engs = [nc.sync, nc.scalar, nc.gpsimd]

            def load_dr_tiles(dram, name):
                tiles = []
                for j in range(KT):
                    t = wp.tile([128, 2, NH], f8, tag=f"{name}{j}",
                                name=f"{name}{j}")
                    _engs[_q[0] % 3].dma_start(out=t, in_=dram[j])
                    _q[0] += 1
                    tiles.append(t)
                return tiles

            # k0-deadline order: the row-group-0/1 tower quarters gate the
            # very first matmuls; mw2 (k0's M2, the next deadline at ~1MB)
            # jumps ahead of the row-group-2/3 quarters, which are only
            # needed once l1_k0 reaches its r=1 waves.
            nc.sync.dma_start(out=w1t5[0:32, :, :], in_=mw1td[:])
            nc.scalar.dma_start(out=w1t5[32:64, :, :], in_=ow1td[:])
            nc.sync.dma_start(out=tow0[0:32, :, :], in_=towd[0])
            nc.scalar.dma_start(out=tow0[32:64, :, :], in_=towd[0])
            mw2 = load_dr_tiles(mw2d, "mw2")  # sync, scalar, gpsimd, sync
            nc.scalar.dma_start(out=w1t5[96:128, :, :], in_=ow1td[:])
            nc.gpsimd.dma_start(out=w1t5[64:96, :, :], in_=mw1td[:])
            nc.gpsimd.dma_start(out=tow0[64:96, :, :], in_=towd[0])
            nc.scalar.dma_start(out=tow0[96:128, :, :], in_=towd[0])
            _q[0] = 1  # resume round-robin on scalar
            mw3 = load_dr_tiles(mw3d, "mw3")
            ow2 = load_dr_tiles(ow2d, "ow2")
            mw1s = load_dr_tiles(mw1sd, "mw1s")
            ow1s = load_dr_tiles(ow1sd, "ow1s")

            ones = smp.tile([128, 1], bf, tag="ones", name="ones")
            nc.vector.memset(ones, 1.0)
            w3b = smp.tile([128, FT], bf, tag="w3b", name="w3b")
            nc.vector.tensor_copy(w3b[:], w3c[:])

            # --- summary double buffer (fp8 pair-tiles).  sA is never
            # read at k=0 (step-0 layer 1 is tower-only), so no
            # initialization is needed. ---
            sA = [[sp.tile([128, 2, RB], f8, tag=f"sA{j}_{r}",
                           name=f"sA{j}_{r}") for r in range(NR)]
                  for j in range(KT)]
            sB = [[sp.tile([128, 2, RB], f8, tag=f"sB{j}_{r}",
                           name=f"sB{j}_{r}") for r in range(NR)]
                  for j in range(KT)]

            # --- product accumulators ---
            pacc = []
            for r in range(NR):
                t = smp.tile([1, RB], f32, tag=f"pacc{r}", name=f"pacc{r}")
                nc.vector.memset(t, 1.0)
                pacc.append(t)

            # bias column: layer l in {0:Mb1 1:Mb2 2:Mb3 3:Ob1 4:Ob2} at
            # l*8+m; step-0 fused biases (b + agg@W1s) at 40+m (M), 48+m (O)
            def relu_epilogue(ot, ps, bias_col, m):
                """Bias+relu out of PSUM; alternate ACT/DVE by m so neither
                engine head-of-line-blocks the PE's psum bank rotation."""
                bias = ball[:, ds(bias_col, 1)]
                if m % 2 == 0:
                    nc.scalar.activation(ot, ps[:], Relu, bias=bias)
                else:
                    nc.vector.tensor_scalar(ot, ps[:], bias, 0.0, Add,
                                            mybir.AluOpType.max)

            def pair_tiles(tag):
                return [[ap.tile([128, 2, RB], f8, tag=tag, name=tag)
                         for _ in range(NR)] for _ in range(KT)]

            def l1_fused(scur, tow_t, include_m=True, deferred=()):
                """Fused M/O layer 1.  Per output tile m: all DoubleRow
                summary matmuls (stationary reused across the NR row
                blocks), then the contraction-64 tower closers with M on
                PE rows 0-63 and O on rows 64-127, issued adjacently so
                each M/O pair streams concurrently.  `deferred` is a list
                of closures (the previous step's DVE head-chain ops),
                drained ~4 per m-iteration so they interleave with this
                layer's epilogues in the DVE FIFO instead of forming a
                9us burst that blocks PSUM bank release."""
                m1p = pair_tiles("m1p") if include_m else None
                h1p = pair_tiles("h1p")
                dq = list(deferred)
                for m in range(FT):
                    psm = [pp.tile([128, RB], f32, tag="ps", name="psm")
                           for _ in range(NR)] if include_m else None
                    pso = [pp.tile([128, RB], f32, tag="ps", name="pso")
                           for _ in range(NR)]
                    for j in range(KT):
                        if include_m:
                            for r in range(NR):
                                nc.tensor.matmul(
                                    psm[r][:], mw1s[j][:, :, ts(m, 128)],
                                    scur[j][r][:], start=(j == 0),
                                    stop=False, perf_mode=DR)
                        for r in range(NR):
                            nc.tensor.matmul(
                                pso[r][:], ow1s[j][:, :, ts(m, 128)],
                                scur[j][r][:], start=(j == 0),
                                stop=False, perf_mode=DR)
                    for r in range(NR):
                        mb, ob = 64 * r, 64 * r + 32
                        if include_m:
                            nc.tensor.matmul(
                                psm[r][:], w1t5[mb:mb + 32, :, ts(m, 128)],
                                tow_t[mb:mb + 32, :, ts(r, RB)],
                                start=False, stop=True, perf_mode=DR,
                                tile_position=(mb, 0))
                        nc.tensor.matmul(
                            pso[r][:], w1t5[ob:ob + 32, :, ts(m, 128)],
                            tow_t[ob:ob + 32, :, ts(r, RB)],
                            start=False, stop=True, perf_mode=DR,
                            tile_position=(ob, 0))
                    q, i = divmod(m, 2)
                    for r in range(NR):
                        if include_m:
                            relu_epilogue(m1p[q][r][:, i, :], psm[r], m, m)
                        relu_epilogue(h1p[q][r][:, i, :], pso[r], 24 + m,
                                      m + 1)
                    take = (len(dq) + FT - 1 - m) // (FT - m)
                    for _ in range(take):
                        dq.pop(0)()
                return m1p, h1p

            def l1_k0(tow_t):
                """Step 0: summary is the broadcast aggregate, folded into
                the bias on the host, so layer 1 is just the tower matmul."""
                m1p = pair_tiles("m1p")
                h1p = pair_tiles("h1p")
                for m in range(FT):
                    psm = [pp.tile([128, RB], f32, tag="ps", name="psm")
                           for _ in range(NR)]
                    pso = [pp.tile([128, RB], f32, tag="ps", name="pso")
                           for _ in range(NR)]
                    for r in range(NR):
                        mb, ob = 64 * r, 64 * r + 32
                        nc.tensor.matmul(
                            psm[r][:], w1t5[mb:mb + 32, :, ts(m, 128)],
                            tow_t[mb:mb + 32, :, ts(r, RB)],
                            start=True, stop=True, perf_mode=DR,
                            tile_position=(mb, 0))
                        nc.tensor.matmul(
                            pso[r][:], w1t5[ob:ob + 32, :, ts(m, 128)],
                            tow_t[ob:ob + 32, :, ts(r, RB)],
                            start=True, stop=True, perf_mode=DR,
                            tile_position=(ob, 0))
                    q, i = divmod(m, 2)
                    for r in range(NR):
                        relu_epilogue(m1p[q][r][:, i, :], psm[r], 40 + m, m)
                        relu_epilogue(h1p[q][r][:, i, :], pso[r], 48 + m,
                                      m + 1)
                return m1p, h1p

            def layer_dr(rhs, ws, writer):
                """1024x1024 DoubleRow layer: per output tile m, KT
                contraction matmuls x NR row blocks, stationary reused
                across row blocks."""
                for m in range(FT):
                    pss = [pp.tile([128, RB], f32, tag="ps", name="ps")
                           for _ in range(NR)]
                    for j in range(KT):
                        for r in range(NR):
                            nc.tensor.matmul(
                                pss[r][:], ws[j][:, :, ts(m, 128)],
                                rhs[j][r][:], start=(j == 0),
                                stop=(j == KT - 1), perf_mode=DR)
                    for r in range(NR):
                        writer(m, r, pss[r])

            def flush_zjobs(zjobs):
                # both row blocks' [1, RB] reduces share one PSUM bank at
                # partitions 0 / 32 (tile_position col groups), freeing a
                # bank for the main psum pool.  (DoubleRow variants of this
                # reduce don't work: small/offset dst partition counts fail
                # walrus's s3d3_mm_valid_dst_partition check, and an fp8
                # hi/lo split of g via mixed-dtype DVE tensor_tensor gave
                # 5.5e-2 rel err on HW — sim/HW divergence.)
                if not zjobs:
                    return
                zt = zp.tile([33, RB], f32, tag="z", name="zps")
                for gb, r in zjobs:
                    zv = zt[ds(32 * r, 1), :]
                    nc.tensor.matmul(zv, ones[:], gb[:],
                                     start=True, stop=True,
                                     tile_position=(0, 32 * r))
                    pr = smp.tile([1, RB], f32, tag=f"pr{r}", name=f"pr{r}")
                    nc.scalar.activation(pr[:], zv, Sigmoid, bias=ob3[:])
                    nc.vector.tensor_mul(pacc[r][:], pacc[r][:], pr[:])

            def head_chain_ops(h2, r, zjobs):
                """The k<K-1 output head as a list of closures: mult/add
                chain on the DVE, drained interleaved into the next step's
                layer-1 loop."""
                st = {}
                ops = []

                def op0():
                    g = zw.tile([128, RB], f32, tag="g", name="g")
                    nc.vector.tensor_scalar(
                        g[:], h2[0][r][:], w3c[:, ds(0, 1)], None, Mult)
                    st["g"] = g
                ops.append(op0)
                for i in range(1, FT):
                    def opm(i=i):
                        t = zw.tile([128, RB], f32, tag="t", name="t",
                                    bufs=3)
                        nc.vector.tensor_scalar(
                            t[:], h2[i][r][:], w3c[:, ds(i, 1)], None, Mult)
                        st["t"] = t

                    def opa():
                        nc.vector.tensor_tensor(st["g"][:], st["g"][:],
                                                st["t"][:], Add)
                    ops += [opm, opa]

                def opc():
                    gb = zw.tile([128, RB], bf, tag="gb", name="gb", bufs=4)
                    nc.vector.tensor_copy(gb[:], st["g"][:])
                    zjobs.append((gb, r))
                ops.append(opc)
                return ops

            def o2_final(h1):
                """Final-step O2 + output head.  The head g = sum_i
                h2_i * w3_i runs as 8 accumulating [128,1]-stationary bf16
                matmuls per row block on the (otherwise idle) PE, emitted
                after both row blocks' DoubleRow streams, so the
                post-matmul tail is one epilogue + ~3.5us of head matmuls
                instead of a ~12us DVE mult/add-tree chain."""
                h2o = [[None] * FT for _ in range(NR)]
                for r in range(NR):
                    for m in range(FT):
                        ps = pp.tile([128, RB], f32, tag="ps", name="ps")
                        for j in range(KT):
                            nc.tensor.matmul(
                                ps[:], ow2[j][:, :, ts(m, 128)],
                                h1[j][r][:], start=(j == 0),
                                stop=(j == KT - 1), perf_mode=DR)
                        h2t = ap.tile([128, RB], bf, tag="h2", name="h2")
                        relu_epilogue(h2t[:], ps, 32 + m, m)
                        h2o[r][m] = h2t
                zt = zp.tile([33, RB], f32, tag="z", name="zfin",
                             uniquify=False)
                for r in range(NR):
                    zv = zt[ds(32 * r, 1), :]
                    for m in range(FT):
                        nc.tensor.matmul(
                            zv, w3b[:, ds(m, 1)], h2o[r][m][:],
                            start=(m == 0), stop=(m == FT - 1),
                            tile_position=(0, 32 * r))
                    pr = smp.tile([1, RB], f32, tag=f"pr{r}", name=f"pr{r}")
                    nc.scalar.activation(pr[:], zv, Sigmoid, bias=ob3[:])
                    nc.vector.tensor_mul(pacc[r][:], pacc[r][:], pr[:])

            scur, snxt = sA, sB
            zjobs = []
            pending = []
            for k in range(K):
                if k == 0:
                    tow_t = tow0
                else:
                    tow_t = twp.tile([128, 2, R], f8, tag="tow", name="tow")
                    for b in range(4):
                        nc.gpsimd.dma_start(
                            out=tow_t[32 * b:32 * b + 32, :, :],
                            in_=towd[k])

                if k == 0:
                    m1, h1 = l1_k0(tow_t)
                elif k == K - 1:
                    # the final scan carry is discarded by the reference, so
                    # the last step's M branch (M1/M2/M3) is dead code
                    _, h1 = l1_fused(scur, tow_t, include_m=False,
                                     deferred=pending)
                    pending = []
                    m1 = None
                else:
                    m1, h1 = l1_fused(scur, tow_t, deferred=pending)
                    pending = []

                if k < K - 1:
                    m2p = pair_tiles("m2p")

                    def w_m2(m, r, ps):
                        q, i = divmod(m, 2)
                        relu_epilogue(m2p[q][r][:, i, :], ps, 8 + m, m)

                    layer_dr(m1, mw2, w_m2)
                    # previous step's output head (its DVE reduce is long
                    # done, so the sigmoid never head-of-line-blocks the
                    # ACT queue)
                    flush_zjobs(zjobs)
                    zjobs = []

                    def w_m3(m, r, ps):
                        q, i = divmod(m, 2)
                        relu_epilogue(snxt[q][r][:, i, :], ps, 16 + m, m)

                    layer_dr(m2p, mw3, w_m3)
                else:
                    flush_zjobs(zjobs)
                    zjobs = []

                if k < K - 1:
                    h2 = [[None] * NR for _ in range(FT)]

                    def w_o2(m, r, ps):
                        t = ap.tile([128, RB], bf, tag="h2", name="h2")
                        relu_epilogue(t[:], ps, 32 + m, m)
                        h2[m][r] = t

                    layer_dr(h1, ow2, w_o2)
                    # g = sum_i h2_i * w3_i on the DVE (per-partition
                    # scalars), reduced across partitions next step by a
                    # ones-matmul.  Emission is deferred into the next
                    # step's layer-1 loop (chains for r=0/r=1 interleaved).
                    c0 = head_chain_ops(h2, 0, zjobs)
                    c1 = head_chain_ops(h2, 1, zjobs)
                    pending = [op for pair in zip(c0, c1) for op in pair]
                else:
                    o2_final(h1)

                scur, snxt = snxt, scur

            for r in range(NR):
                nc.sync.dma_start(out=outd[:, ts(r, RB)], in_=pacc[r][:])

    nc.finalize()
    return nc


def _get_nc():
    global _BUILT
    if _BUILT is None:
        _BUILT = _build()
    return _BUILT


def _dr_quant(W):
    """[NH, NH] f32 -> [KT, 128, 2, NH] e4m3 DoubleRow interleave:
    out[j, p, i, m] = W[256j + 128i + p, m]."""
    return np.ascontiguousarray(
        W.reshape(KT, 2, 128, NH).transpose(0, 2, 1, 3)).astype(F8)


def _dr_pair32(W):
    """[64, X] f32 -> [32, 2, X] e4m3: out[p, i, x] = W[32i + p, x]."""
    return np.ascontiguousarray(
        W.reshape(2, 32, -1).transpose(1, 0, 2)).astype(F8)


def _prep_inputs(inputs):
    f32 = np.float32
    towers = np.asarray(inputs["towers"], dtype=f32)
    agg = np.asarray(inputs["aggregate"], dtype=f32)
    MW1 = np.asarray(inputs["MW1"], dtype=f32)
    OW1 = np.asarray(inputs["OW1"], dtype=f32)

    def col8(v):
        return np.asarray(v, f32).reshape(FT, 128).T

    # step-0 biases with the broadcast-aggregate layer-1 contribution folded
    b0m = np.asarray(inputs["Mb1"], f32) + agg[0] @ MW1[:NH]
    b0o = np.asarray(inputs["Ob1"], f32) + agg[0] @ OW1[:NH]

    shared = {
        "mw1s": _dr_quant(MW1[:NH]),
        "mw1t": _dr_pair32(MW1[NH:]),
        "mw2": _dr_quant(np.asarray(inputs["MW2"], f32)),
        "mw3": _dr_quant(np.asarray(inputs["MW3"], f32)),
        "ow1s": _dr_quant(OW1[:NH]),
        "ow1t": _dr_pair32(OW1[NH:]),
        "ow2": _dr_quant(np.asarray(inputs["OW2"], f32)),
        "w3c": np.ascontiguousarray(col8(np.asarray(inputs["OW3"], f32))),
        "ball": np.ascontiguousarray(np.concatenate(
            [col8(inputs[b]) for b in ("Mb1", "Mb2", "Mb3", "Ob1", "Ob2")]
            + [col8(b0m), col8(b0o)], axis=1)),
        "ob3": np.asarray(inputs["Ob3"], f32).reshape(1, 1),
    }
    in_maps = []
    for c in range(N_CORES):
        tc_ = towers[c * R:(c + 1) * R]          # (R, K, NI)
        t = tc_.transpose(1, 2, 0)               # (K, NI, R)
        towT = np.ascontiguousarray(
            t.reshape(K, 2, NI // 2, R).transpose(0, 2, 1, 3)).astype(F8)
        in_maps.append({"tow": towT, **shared})
    return in_maps


def _run(inputs, trace=False):
    nc = _get_nc()
    in_maps = _prep_inputs(inputs)
    res = run_bass_kernel_spmd(nc, in_maps, list(range(N_CORES)), trace=trace)
    out = np.concatenate([res.results[c]["out"][0] for c in range(N_CORES)])
    return out.astype(np.float32), res


def kernel(**inputs):
    out, _ = _run(inputs, trace=False)
    return out


# revision 61
# speedup vs baseline: 1.0004x; 1.0004x over previous
"""Trainium2 Bass kernel for nn_BottomUpNet (dense_mlp).

Reference computation (per row n of N=8192, fully independent across rows):
    summary = aggregate (broadcast)                   # (1024,)
    for k in 0..15:
        x = [summary, towers[n, k, :]]                # (1088,)
        h = relu(x @ OW1 + Ob1); h = relu(h @ OW2 + Ob2)
        pred_k = sigmoid(h @ OW3 + Ob3)
        m = relu(x @ MW1 + Mb1); m = relu(m @ MW2 + Mb2); m = relu(m @ MW3 + Mb3)
        summary = m
    out[n] = prod_k pred_k

Strategy: data-parallel over N across 8 cores (1024 rows each), weights
replicated.  Activations are feature-major ([feature partition, row free])
so weight matrices serve directly as the stationary matmul operand and no
on-chip transposes are needed.

All matmuls run in fp8-e4m3 with perf_mode=DoubleRow: both operands carry
contraction pairs [K, 2, free] so each matmul instruction reduces 2*K rows
(2 fp8 weights per PE cell), ~1.7x the bf16 streaming rate.  The five
1024-contraction layers per step (M1s, M2, M3, O1s, O2) use [128, 2, *]
operands; weights are pre-interleaved on the host into [ktile, 128, 2,
NH]; activations feeding these matmuls are written by the epilogues as
fp8 pair-tiles [128, 2, 512].  The 64-wide tower matmuls are [32, 2, *]
DoubleRow closers of the layer-1 accumulation groups.  End-to-end rel err
vs the f32 reference ~1.1e-2 (fp8 quantization noise; the e4m3 denormal
range covers the small uniform weights acceptably, so no weight scaling
is needed and bias+relu epilogues keep their single-op form).  f32 PSUM
accumulation throughout; the output head stays bf16/f32.

Perf structure (measured 1.172 ms vs 2.249 ms for the best bf16 version;
PE busy ~96%, pinned on the DoubleRow streaming rate of ~216ns per
256x128x512 matmul):
  - loop order is m-outer / row-block-inner so each DoubleRow stationary
    tile (256x128 weight block) is reused by NR=2 matmuls, halving
    LDWEIGHTS traffic (DoubleRow weight loads are 2x the columns).
  - the four per-m layer-1 tower closers (M/O branch x 2 row blocks) are
    placed in the four disjoint 32-row PE row groups (tower data + weights
    duplicated across partition quarters), so all four stream concurrently
    in the systolic array (~216ns for the wave).
  - step 0's summary is the broadcast aggregate, identical for all rows:
    its layer-1 contribution agg @ W1s is folded into the step-0 bias on
    the host, so step 0's layer 1 is just the tower matmuls.
  - the 1024->1 output head is a DVE per-partition multiply/add tree
    (g = sum_i h2_i * w3_i) plus a single ones-vector matmul for the
    cross-partition reduce; the chain's DVE ops are emitted as deferred
    closures drained ~4 per m-iteration of the NEXT step's layer-1 loop,
    so they interleave with the epilogues in the DVE FIFO instead of
    forming a ~9us burst that delays PSUM bank release (which showed up
    as ~150ns stalls on group-opening matmuls).  The sigmoid + product-
    accumulate are deferred likewise.
  - the final step's M branch (M1/M2/M3) is skipped entirely: the
    reference discards the last scan carry, so that summary is dead; the
    final step's head runs as 8 accumulating [128,1]-stationary bf16
    matmuls per row block on the otherwise-idle PE (emitted after both
    row blocks' DoubleRow streams), so the post-matmul tail is ~5us
    instead of a ~12us serialized DVE mult/add-tree chain.
  - relu epilogues alternate between the scalar and vector engines,
    staggered so each m-iteration hits both engines once.
  - both [1, 512] head reduces share one PSUM bank at partitions 0/32
    (explicit tile_position), freeing the 8th bank for the matmul pool.
  - weight DMAs round-robin over the three DGE queues in first-use order;
    step 0 needs only the tower weights + biases + step-0 towers, which
    are issued first so the PE starts at ~10us.

Known residual costs (~90us over the ~1.09ms matmul-stream floor):
LDWEIGHTS exposure when switching between 32-row and 128-row stationaries
around each tower wave (~280ns per m-group; a full-row LDWEIGHTS cannot
be pulled ahead past in-flight partial-row matmuls and vice versa),
~7us of weight-DMA gating during steps 0-1 (5MB of fp8 weights over
three ~45GB/s queues), and chip-state variance: under sustained load the
chip drops PE 2.4->2.0GHz (P0), inflating wall time ~20% run-to-run.
"""

import numpy as np
import ml_dtypes

import concourse.bacc as bacc
import concourse.mybir as mybir
import concourse.tile as tile
from concourse.bass import ts, ds
from concourse.bass_utils import run_bass_kernel_spmd

BF16 = ml_dtypes.bfloat16
F8 = ml_dtypes.float8_e4m3

N_CORES = 8
N = 8192
K = 16
NI = 64          # tower features per step
NH = 1024        # hidden width
FT = NH // 128   # feature tiles (8)
KT = NH // 256   # DoubleRow contraction tiles (4)
R = N // N_CORES  # rows per core (1024)
RB = 512         # row block (matmul moving dim / one PSUM bank)
NR = R // RB     # row blocks per core (2)

_BUILT = None


def _build():
    nc = bacc.Bacc("TRN2", target_bir_lowering=False, debug=False,
                   num_devices=N_CORES)
    f32 = mybir.dt.float32
    bf = mybir.dt.bfloat16
    f8 = mybir.dt.float8e4
    DR = mybir.MatmulPerfMode.DoubleRow

    towd = nc.declare_dram_parameter("tow", [K, NI // 2, 2, R], f8,
                                     isOutput=False)
    mw1sd = nc.declare_dram_parameter("mw1s", [KT, 128, 2, NH], f8,
                                      isOutput=False)
    mw1td = nc.declare_dram_parameter("mw1t", [NI // 2, 2, NH], f8,
                                      isOutput=False)
    mw2d = nc.declare_dram_parameter("mw2", [KT, 128, 2, NH], f8,
                                     isOutput=False)
    mw3d = nc.declare_dram_parameter("mw3", [KT, 128, 2, NH], f8,
                                     isOutput=False)
    ow1sd = nc.declare_dram_parameter("ow1s", [KT, 128, 2, NH], f8,
                                      isOutput=False)
    ow1td = nc.declare_dram_parameter("ow1t", [NI // 2, 2, NH], f8,
                                      isOutput=False)
    ow2d = nc.declare_dram_parameter("ow2", [KT, 128, 2, NH], f8,
                                     isOutput=False)
    w3cd = nc.declare_dram_parameter("w3c", [128, FT], f32, isOutput=False)
    balld = nc.declare_dram_parameter("ball", [128, 56], f32, isOutput=False)
    ob3d = nc.declare_dram_parameter("ob3", [1, 1], f32, isOutput=False)
    outd = nc.declare_dram_parameter("out", [1, R], f32, isOutput=True)

    Relu = mybir.ActivationFunctionType.Relu
    Sigmoid = mybir.ActivationFunctionType.Sigmoid
    Identity = mybir.ActivationFunctionType.Identity
    Add = mybir.AluOpType.add
    Mult = mybir.AluOpType.mult

    with tile.TileContext(nc) as tc:
        with (
            tc.tile_pool(name="weights", bufs=1) as wp,
            tc.tile_pool(name="summary", bufs=1) as sp,
            tc.tile_pool(name="acts", bufs=16) as ap,
            tc.tile_pool(name="tow", bufs=4) as twp,
            tc.tile_pool(name="small", bufs=1) as smp,
            tc.tile_pool(name="zwork", bufs=2) as zw,
            tc.tile_pool(name="psum", bufs=7, space="PSUM") as pp,
            tc.tile_pool(name="zpsum", bufs=1, space="PSUM") as zp,
        ):
            # --- small/early tiles on the gpsimd SW queue; step 0 only
            # needs the tower weights + biases to start.  The step-0 tower
            # weights + tower data go first on the two HW DGE queues, and
            # the weights are spread over four queues in deadline order
            # (step 0: M2, M3, O2; step 1 adds M1s, O1s on vector/tensor
    # queues so they load in parallel with the step-0 weights). ---
            ball = smp.tile([128, 56], f32, tag="ball", name="ball")
            nc.gpsimd.dma_start(out=ball, in_=balld[:])
            ob3 = smp.tile([1, 1], f32, tag="ob3", name="ob3")
            nc.gpsimd.dma_start(out=ob3, in_=ob3d[:])
            w3c = smp.tile([128, FT], f32, tag="w3c", name="w3c")
            nc.gpsimd.dma_start(out=w3c, in_=w3cd[:])
            # tower weights as DoubleRow pairs, duplicated across all four
            # 32-row PE row groups (M r0 / O r0 / M r1 / O r1) so the four
            # per-m tower matmuls all stream concurrently
            w1t5 = wp.tile([128, 2, NH], f8, tag="w1t5", name="w1t5")
            tow0 = twp.tile([128, 2, R], f8, tag="tow", name="tow")

            _q = [0]
            _engs = [nc.sync, nc.scalar, nc.gpsimd]

            def load_dr_tiles(dram, name):
                tiles = []
                for j in range(KT):
                    t = wp.tile([128, 2, NH], f8, tag=f"{name}{j}",
                                name=f"{name}{j}")
                    _engs[_q[0] % 3].dma_start(out=t, in_=dram[j])
                    _q[0] += 1
                    tiles.append(t)
                return tiles

            # k0-deadline order: the row-group-0/1 tower quarters gate the
            # very first matmuls; mw2 (k0's M2, the next deadline at ~1MB)
            # jumps ahead of the row-group-2/3 quarters, which are only
            # needed once l1_k0 reaches its r=1 waves.
            nc.sync.dma_start(out=w1t5[0:32, :, :], in_=mw1td[:])
            nc.scalar.dma_start(out=w1t5[32:64, :, :], in_=ow1td[:])
            nc.sync.dma_start(out=tow0[0:32, :, :], in_=towd[0])
            nc.scalar.dma_start(out=tow0[32:64, :, :], in_=towd[0])
            mw2 = load_dr_tiles(mw2d, "mw2")  # sync, scalar, gpsimd, sync
            nc.scalar.dma_start(out=w1t5[96:128, :, :], in_=ow1td[:])
            nc.gpsimd.dma_start(out=w1t5[64:96, :, :], in_=mw1td[:])
            nc.gpsimd.dma_start(out=tow0[64:96, :, :], in_=towd[0])
            nc.scalar.dma_start(out=tow0[96:128, :, :], in_=towd[0])
            _q[0] = 1  # resume round-robin on scalar
            mw3 = load_dr_tiles(mw3d, "mw3")
            ow2 = load_dr_tiles(ow2d, "ow2")
            mw1s = load_dr_tiles(mw1sd, "mw1s")
            ow1s = load_dr_tiles(ow1sd, "ow1s")

            ones = smp.tile([128, 1], bf, tag="ones", name="ones")
            nc.vector.memset(ones, 1.0)
            w3b = smp.tile([128, FT], bf, tag="w3b", name="w3b")
            nc.vector.tensor_copy(w3b[:], w3c[:])

            # --- summary double buffer (fp8 pair-tiles).  sA is never
            # read at k=0 (step-0 layer 1 is tower-only), so no
            # initialization is needed. ---
            sA = [[sp.tile([128, 2, RB], f8, tag=f"sA{j}_{r}",
                           name=f"sA{j}_{r}") for r in range(NR)]
                  for j in range(KT)]
            sB = [[sp.tile([128, 2, RB], f8, tag=f"sB{j}_{r}",
                           name=f"sB{j}_{r}") for r in range(NR)]
                  for j in range(KT)]

            # --- product accumulators ---
            pacc = []
            for r in range(NR):
                t = smp.tile([1, RB], f32, tag=f"pacc{r}", name=f"pacc{r}")
                nc.vector.memset(t, 1.0)
                pacc.append(t)

            # bias column: layer l in {0:Mb1 1:Mb2 2:Mb3 3:Ob1 4:Ob2} at
            # l*8+m; step-0 fused biases (b + agg@W1s) at 40+m (M), 48+m (O)
            def relu_epilogue(ot, ps, bias_col, m):
                """Bias+relu out of PSUM; alternate ACT/DVE by m so neither
                engine head-of-line-blocks the PE's psum bank rotation."""
                bias = ball[:, ds(bias_col, 1)]
                if m % 2 == 0:
                    nc.scalar.activation(ot, ps[:], Relu, bias=bias)
                else:
                    nc.vector.tensor_scalar(ot, ps[:], bias, 0.0, Add,
                                            mybir.AluOpType.max)

            def pair_tiles(tag):
                return [[ap.tile([128, 2, RB], f8, tag=tag, name=tag)
                         for _ in range(NR)] for _ in range(KT)]

            def l1_fused(scur, tow_t, include_m=True, deferred=()):
                """Fused M/O layer 1.  Per output tile m: all DoubleRow
                summary matmuls (stationary reused across the NR row
                blocks), then the contraction-64 tower closers with M on
                PE rows 0-63 and O on rows 64-127, issued adjacently so
                each M/O pair streams concurrently.  `deferred` is a list
                of closures (the previous step's DVE head-chain ops),
                drained ~4 per m-iteration so they interleave with this
                layer's epilogues in the DVE FIFO instead of forming a
                9us burst that blocks PSUM bank release."""
                m1p = pair_tiles("m1p") if include_m else None
                h1p = pair_tiles("h1p")
                dq = list(deferred)
                for m in range(FT):
                    psm = [pp.tile([128, RB], f32, tag="ps", name="psm")
                           for _ in range(NR)] if include_m else None
                    pso = [pp.tile([128, RB], f32, tag="ps", name="pso")
                           for _ in range(NR)]
                    for j in range(KT):
                        if include_m:
                            for r in range(NR):
                                nc.tensor.matmul(
                                    psm[r][:], mw1s[j][:, :, ts(m, 128)],
                                    scur[j][r][:], start=(j == 0),
                                    stop=False, perf_mode=DR)
                        for r in range(NR):
                            nc.tensor.matmul(
                                pso[r][:], ow1s[j][:, :, ts(m, 128)],
                                scur[j][r][:], start=(j == 0),
                                stop=False, perf_mode=DR)
                    for r in range(NR):
                        mb, ob = 64 * r, 64 * r + 32
                        if include_m:
                            nc.tensor.matmul(
                                psm[r][:], w1t5[mb:mb + 32, :, ts(m, 128)],
                                tow_t[mb:mb + 32, :, ts(r, RB)],
                                start=False, stop=True, perf_mode=DR,
                                tile_position=(mb, 0))
                        nc.tensor.matmul(
                            pso[r][:], w1t5[ob:ob + 32, :, ts(m, 128)],
                            tow_t[ob:ob + 32, :, ts(r, RB)],
                            start=False, stop=True, perf_mode=DR,
                            tile_position=(ob, 0))
                    q, i = divmod(m, 2)
                    # M epilogues first: the next layer's j=3 matmuls wait
                    # on the last m1 pair, while h1 isn't read until O2
                    if include_m:
                        for r in range(NR):
                            relu_epilogue(m1p[q][r][:, i, :], psm[r], m, m)
                    for r in range(NR):
                        relu_epilogue(h1p[q][r][:, i, :], pso[r], 24 + m,
                                      m + 1)
                    take = (len(dq) + FT - 1 - m) // (FT - m)
                    for _ in range(take):
                        dq.pop(0)()
                return m1p, h1p

            def l1_k0(tow_t):
                """Step 0: summary is the broadcast aggregate, folded into
                the bias on the host, so layer 1 is just the tower matmul."""
                m1p = pair_tiles("m1p")
                h1p = pair_tiles("h1p")
                for m in range(FT):
                    psm = [pp.tile([128, RB], f32, tag="ps", name="psm")
                           for _ in range(NR)]
                    pso = [pp.tile([128, RB], f32, tag="ps", name="pso")
                           for _ in range(NR)]
                    for r in range(NR):
                        mb, ob = 64 * r, 64 * r + 32
                        nc.tensor.matmul(
                            psm[r][:], w1t5[mb:mb + 32, :, ts(m, 128)],
                            tow_t[mb:mb + 32, :, ts(r, RB)],
                            start=True, stop=True, perf_mode=DR,
                            tile_position=(mb, 0))
                        nc.tensor.matmul(
                            pso[r][:], w1t5[ob:ob + 32, :, ts(m, 128)],
                            tow_t[ob:ob + 32, :, ts(r, RB)],
                            start=True, stop=True, perf_mode=DR,
                            tile_position=(ob, 0))
                    q, i = divmod(m, 2)
                    for r in range(NR):
                        relu_epilogue(m1p[q][r][:, i, :], psm[r], 40 + m, m)
                        relu_epilogue(h1p[q][r][:, i, :], pso[r], 48 + m,
                                      m + 1)
                return m1p, h1p

            def layer_dr(rhs, ws, writer):
                """1024x1024 DoubleRow layer: per output tile m, KT
                contraction matmuls x NR row blocks, stationary reused
                across row blocks."""
                for m in range(FT):
                    pss = [pp.tile([128, RB], f32, tag="ps", name="ps")
                           for _ in range(NR)]
                    for j in range(KT):
                        for r in range(NR):
                            nc.tensor.matmul(
                                pss[r][:], ws[j][:, :, ts(m, 128)],
                                rhs[j][r][:], start=(j == 0),
                                stop=(j == KT - 1), perf_mode=DR)
                    for r in range(NR):
                        writer(m, r, pss[r])

            def flush_zjobs(zjobs):
                # both row blocks' [1, RB] reduces share one PSUM bank at
                # partitions 0 / 32 (tile_position col groups), freeing a
                # bank for the main psum pool.  (DoubleRow variants of this
                # reduce don't work: small/offset dst partition counts fail
                # walrus's s3d3_mm_valid_dst_partition check, and an fp8
                # hi/lo split of g via mixed-dtype DVE tensor_tensor gave
                # 5.5e-2 rel err on HW — sim/HW divergence.)
                if not zjobs:
                    return
                zt = zp.tile([33, RB], f32, tag="z", name="zps")
                for gb, r in zjobs:
                    zv = zt[ds(32 * r, 1), :]
                    nc.tensor.matmul(zv, ones[:], gb[:],
                                     start=True, stop=True,
                                     tile_position=(0, 32 * r))
                    pr = smp.tile([1, RB], f32, tag=f"pr{r}", name=f"pr{r}")
                    nc.scalar.activation(pr[:], zv, Sigmoid, bias=ob3[:])
                    nc.vector.tensor_mul(pacc[r][:], pacc[r][:], pr[:])

            def head_chain_ops(h2, r, zjobs):
                """The k<K-1 output head as a list of closures: mult/add
                chain on the DVE, drained interleaved into the next step's
                layer-1 loop."""
                st = {}
                ops = []

                def op0():
                    g = zw.tile([128, RB], f32, tag="g", name="g")
                    nc.vector.tensor_scalar(
                        g[:], h2[0][r][:], w3c[:, ds(0, 1)], None, Mult)
                    st["g"] = g
                ops.append(op0)
                for i in range(1, FT):
                    def opm(i=i):
                        t = zw.tile([128, RB], f32, tag="t", name="t",
                                    bufs=3)
                        nc.vector.tensor_scalar(
                            t[:], h2[i][r][:], w3c[:, ds(i, 1)], None, Mult)
                        st["t"] = t

                    def opa():
                        nc.vector.tensor_tensor(st["g"][:], st["g"][:],
                                                st["t"][:], Add)
                    ops += [opm, opa]

                def opc():
                    gb = zw.tile([128, RB], bf, tag="gb", name="gb", bufs=4)
                    nc.vector.tensor_copy(gb[:], st["g"][:])
                    zjobs.append((gb, r))
                ops.append(opc)
                return ops

            def o2_final(h1):
                """Final-step O2 + output head.  The head g = sum_i
                h2_i * w3_i runs as 8 accumulating [128,1]-stationary bf16
                matmuls per row block on the (otherwise idle) PE, emitted
                after both row blocks' DoubleRow streams, so the
                post-matmul tail is one epilogue + ~3.5us of head matmuls
                instead of a ~12us DVE mult/add-tree chain."""
                h2o = [[None] * FT for _ in range(NR)]
                for r in range(NR):
                    for m in range(FT):
                        ps = pp.tile([128, RB], f32, tag="ps", name="ps")
                        for j in range(KT):
                            nc.tensor.matmul(
                                ps[:], ow2[j][:, :, ts(m, 128)],
                                h1[j][r][:], start=(j == 0),
                                stop=(j == KT - 1), perf_mode=DR)
                        h2t = ap.tile([128, RB], bf, tag="h2", name="h2")
                        relu_epilogue(h2t[:], ps, 32 + m, m)
                        h2o[r][m] = h2t
                zt = zp.tile([33, RB], f32, tag="z", name="zfin",
                             uniquify=False)
                for r in range(NR):
                    zv = zt[ds(32 * r, 1), :]
                    for m in range(FT):
                        nc.tensor.matmul(
                            zv, w3b[:, ds(m, 1)], h2o[r][m][:],
                            start=(m == 0), stop=(m == FT - 1),
                            tile_position=(0, 32 * r))
                    pr = smp.tile([1, RB], f32, tag=f"pr{r}", name=f"pr{r}")
                    nc.scalar.activation(pr[:], zv, Sigmoid, bias=ob3[:])
                    nc.vector.tensor_mul(pacc[r][:], pacc[r][:], pr[:])

            scur, snxt = sA, sB
            zjobs = []
            pending = []
            for k in range(K):
                if k == 0:
                    tow_t = tow0
                else:
                    tow_t = twp.tile([128, 2, R], f8, tag="tow", name="tow")
                    for b in range(4):
                        nc.gpsimd.dma_start(
                            out=tow_t[32 * b:32 * b + 32, :, :],
                            in_=towd[k])

                if k == 0:
                    m1, h1 = l1_k0(tow_t)
                elif k == K - 1:
                    # the final scan carry is discarded by the reference, so
                    # the last step's M branch (M1/M2/M3) is dead code
                    _, h1 = l1_fused(scur, tow_t, include_m=False,
                                     deferred=pending)
                    pending = []
                    m1 = None
                else:
                    m1, h1 = l1_fused(scur, tow_t, deferred=pending)
                    pending = []

                if k < K - 1:
                    m2p = pair_tiles("m2p")

                    def w_m2(m, r, ps):
                        q, i = divmod(m, 2)
                        relu_epilogue(m2p[q][r][:, i, :], ps, 8 + m, m)

                    layer_dr(m1, mw2, w_m2)
                    # previous step's output head (its DVE reduce is long
                    # done, so the sigmoid never head-of-line-blocks the
                    # ACT queue)
                    flush_zjobs(zjobs)
                    zjobs = []

                    def w_m3(m, r, ps):
                        q, i = divmod(m, 2)
                        relu_epilogue(snxt[q][r][:, i, :], ps, 16 + m, m)

                    layer_dr(m2p, mw3, w_m3)
                else:
                    flush_zjobs(zjobs)
                    zjobs = []

                if k < K - 1:
                    h2 = [[None] * NR for _ in range(FT)]

                    def w_o2(m, r, ps):
                        t = ap.tile([128, RB], bf, tag="h2", name="h2")
                        relu_epilogue(t[:], ps, 32 + m, m)
                        h2[m][r] = t

                    layer_dr(h1, ow2, w_o2)
                    # g = sum_i h2_i * w3_i on the DVE (per-partition
                    # scalars), reduced across partitions next step by a
                    # ones-matmul.  Emission is deferred into the next
                    # step's layer-1 loop (chains for r=0/r=1 interleaved).
                    c0 = head_chain_ops(h2, 0, zjobs)
                    c1 = head_chain_ops(h2, 1, zjobs)
                    pending = [op for pair in zip(c0, c1) for op in pair]
                else:
                    o2_final(h1)

                scur, snxt = snxt, scur

            for r in range(NR):
                nc.sync.dma_start(out=outd[:, ts(r, RB)], in_=pacc[r][:])

    nc.finalize()
    return nc


def _get_nc():
    global _BUILT
    if _BUILT is None:
        _BUILT = _build()
    return _BUILT


def _dr_quant(W):
    """[NH, NH] f32 -> [KT, 128, 2, NH] e4m3 DoubleRow interleave:
    out[j, p, i, m] = W[256j + 128i + p, m]."""
    return np.ascontiguousarray(
        W.reshape(KT, 2, 128, NH).transpose(0, 2, 1, 3)).astype(F8)


def _dr_pair32(W):
    """[64, X] f32 -> [32, 2, X] e4m3: out[p, i, x] = W[32i + p, x]."""
    return np.ascontiguousarray(
        W.reshape(2, 32, -1).transpose(1, 0, 2)).astype(F8)


def _prep_inputs(inputs):
    f32 = np.float32
    towers = np.asarray(inputs["towers"], dtype=f32)
    agg = np.asarray(inputs["aggregate"], dtype=f32)
    MW1 = np.asarray(inputs["MW1"], dtype=f32)
    OW1 = np.asarray(inputs["OW1"], dtype=f32)

    def col8(v):
        return np.asarray(v, f32).reshape(FT, 128).T

    # step-0 biases with the broadcast-aggregate layer-1 contribution folded
    b0m = np.asarray(inputs["Mb1"], f32) + agg[0] @ MW1[:NH]
    b0o = np.asarray(inputs["Ob1"], f32) + agg[0] @ OW1[:NH]

    shared = {
        "mw1s": _dr_quant(MW1[:NH]),
        "mw1t": _dr_pair32(MW1[NH:]),
        "mw2": _dr_quant(np.asarray(inputs["MW2"], f32)),
        "mw3": _dr_quant(np.asarray(inputs["MW3"], f32)),
        "ow1s": _dr_quant(OW1[:NH]),
        "ow1t": _dr_pair32(OW1[NH:]),
        "ow2": _dr_quant(np.asarray(inputs["OW2"], f32)),
        "w3c": np.ascontiguousarray(col8(np.asarray(inputs["OW3"], f32))),
        "ball": np.ascontiguousarray(np.concatenate(
            [col8(inputs[b]) for b in ("Mb1", "Mb2", "Mb3", "Ob1", "Ob2")]
            + [col8(b0m), col8(b0o)], axis=1)),
        "ob3": np.asarray(inputs["Ob3"], f32).reshape(1, 1),
    }
    in_maps = []
    for c in range(N_CORES):
        tc_ = towers[c * R:(c + 1) * R]          # (R, K, NI)
        t = tc_.transpose(1, 2, 0)               # (K, NI, R)
        towT = np.ascontiguousarray(
            t.reshape(K, 2, NI // 2, R).transpose(0, 2, 1, 3)).astype(F8)
        in_maps.append({"tow": towT, **shared})
    return in_maps


def _run(inputs, trace=False):
    nc = _get_nc()
    in_maps = _prep_inputs(inputs)
    res = run_bass_kernel_spmd(nc, in_maps, list(range(N_CORES)), trace=trace)
    out = np.concatenate([res.results[c]["out"][0] for c in range(N_CORES)])
    return out.astype(np.float32), res


def kernel(**inputs):
    out, _ = _run(inputs, trace=False)
    return out


# revision 62
# speedup vs baseline: 1.0112x; 1.0107x over previous
"""Trainium2 Bass kernel for nn_BottomUpNet (dense_mlp).

Reference computation (per row n of N=8192, fully independent across rows):
    summary = aggregate (broadcast)                   # (1024,)
    for k in 0..15:
        x = [summary, towers[n, k, :]]                # (1088,)
        h = relu(x @ OW1 + Ob1); h = relu(h @ OW2 + Ob2)
        pred_k = sigmoid(h @ OW3 + Ob3)
        m = relu(x @ MW1 + Mb1); m = relu(m @ MW2 + Mb2); m = relu(m @ MW3 + Mb3)
        summary = m
    out[n] = prod_k pred_k

Strategy: data-parallel over N across 8 cores (1024 rows each), weights
replicated.  Activations are feature-major ([feature partition, row free])
so weight matrices serve directly as the stationary matmul operand and no
on-chip transposes are needed.

All matmuls run in fp8-e4m3 with perf_mode=DoubleRow: both operands carry
contraction pairs [K, 2, free] so each matmul instruction reduces 2*K rows
(2 fp8 weights per PE cell), ~1.7x the bf16 streaming rate.  The five
1024-contraction layers per step (M1s, M2, M3, O1s, O2) use [128, 2, *]
operands; weights are pre-interleaved on the host into [ktile, 128, 2,
NH]; activations feeding these matmuls are written by the epilogues as
fp8 pair-tiles [128, 2, 512].  The 64-wide tower matmuls are [32, 2, *]
DoubleRow closers of the layer-1 accumulation groups.  End-to-end rel err
vs the f32 reference ~1.1e-2 (fp8 quantization noise; the e4m3 denormal
range covers the small uniform weights acceptably, so no weight scaling
is needed and bias+relu epilogues keep their single-op form).  f32 PSUM
accumulation throughout; the output head stays bf16/f32.

Perf structure (measured 1.172 ms vs 2.249 ms for the best bf16 version;
PE busy ~96%, pinned on the DoubleRow streaming rate of ~216ns per
256x128x512 matmul):
  - loop order is m-outer / row-block-inner so each DoubleRow stationary
    tile (256x128 weight block) is reused by NR=2 matmuls, halving
    LDWEIGHTS traffic (DoubleRow weight loads are 2x the columns).
  - the four per-m layer-1 tower closers (M/O branch x 2 row blocks) are
    placed in the four disjoint 32-row PE row groups (tower data + weights
    duplicated across partition quarters), so all four stream concurrently
    in the systolic array (~216ns for the wave).
  - step 0's summary is the broadcast aggregate, identical for all rows:
    its layer-1 contribution agg @ W1s is folded into the step-0 bias on
    the host, so step 0's layer 1 is just the tower matmuls.
  - the 1024->1 output head is a DVE per-partition multiply/add tree
    (g = sum_i h2_i * w3_i) plus a single ones-vector matmul for the
    cross-partition reduce; the chain's DVE ops are emitted as deferred
    closures drained ~4 per m-iteration of the NEXT step's layer-1 loop,
    so they interleave with the epilogues in the DVE FIFO instead of
    forming a ~9us burst that delays PSUM bank release (which showed up
    as ~150ns stalls on group-opening matmuls).  The sigmoid + product-
    accumulate are deferred likewise.
  - the final step's M branch (M1/M2/M3) is skipped entirely: the
    reference discards the last scan carry, so that summary is dead; the
    final step's head runs as 8 accumulating [128,1]-stationary bf16
    matmuls per row block on the otherwise-idle PE (emitted after both
    row blocks' DoubleRow streams), so the post-matmul tail is ~5us
    instead of a ~12us serialized DVE mult/add-tree chain.
  - relu epilogues alternate between the scalar and vector engines,
    staggered so each m-iteration hits both engines once.
  - both [1, 512] head reduces share one PSUM bank at partitions 0/32
    (explicit tile_position), freeing the 8th bank for the matmul pool.
  - weight DMAs round-robin over the three DGE queues in first-use order;
    step 0 needs only the tower weights + biases + step-0 towers, which
    are issued first so the PE starts at ~10us.

Known residual costs (~90us over the ~1.09ms matmul-stream floor):
LDWEIGHTS exposure when switching between 32-row and 128-row stationaries
around each tower wave (~280ns per m-group; a full-row LDWEIGHTS cannot
be pulled ahead past in-flight partial-row matmuls and vice versa),
~7us of weight-DMA gating during steps 0-1 (5MB of fp8 weights over
three ~45GB/s queues), and chip-state variance: under sustained load the
chip drops PE 2.4->2.0GHz (P0), inflating wall time ~20% run-to-run.
"""

import numpy as np
import ml_dtypes

import concourse.bacc as bacc
import concourse.mybir as mybir
import concourse.tile as tile
from concourse.bass import ts, ds
from concourse.bass_utils import run_bass_kernel_spmd

BF16 = ml_dtypes.bfloat16
F8 = ml_dtypes.float8_e4m3

N_CORES = 8
N = 8192
K = 16
NI = 64          # tower features per step
NH = 1024        # hidden width
FT = NH // 128   # feature tiles (8)
KT = NH // 256   # DoubleRow contraction tiles (4)
R = N // N_CORES  # rows per core (1024)
RB = 512         # row block (matmul moving dim / one PSUM bank)
NR = R // RB     # row blocks per core (2)

_BUILT = None


def _build():
    nc = bacc.Bacc("TRN2", target_bir_lowering=False, debug=False,
                   num_devices=N_CORES)
    f32 = mybir.dt.float32
    bf = mybir.dt.bfloat16
    f8 = mybir.dt.float8e4
    DR = mybir.MatmulPerfMode.DoubleRow

    towd = nc.declare_dram_parameter("tow", [K, NI // 2, 2, R], f8,
                                     isOutput=False)
    mw1sd = nc.declare_dram_parameter("mw1s", [KT, 128, 2, NH], f8,
                                      isOutput=False)
    mw1td = nc.declare_dram_parameter("mw1t", [NI // 2, 2, NH], f8,
                                      isOutput=False)
    mw2d = nc.declare_dram_parameter("mw2", [KT, 128, 2, NH], f8,
                                     isOutput=False)
    mw3d = nc.declare_dram_parameter("mw3", [KT, 128, 2, NH], f8,
                                     isOutput=False)
    ow1sd = nc.declare_dram_parameter("ow1s", [KT, 128, 2, NH], f8,
                                      isOutput=False)
    ow1td = nc.declare_dram_parameter("ow1t", [NI // 2, 2, NH], f8,
                                      isOutput=False)
    ow2d = nc.declare_dram_parameter("ow2", [KT, 128, 2, NH], f8,
                                     isOutput=False)
    w3cd = nc.declare_dram_parameter("w3c", [128, FT], f32, isOutput=False)
    balld = nc.declare_dram_parameter("ball", [128, 56], f32, isOutput=False)
    ob3d = nc.declare_dram_parameter("ob3", [1, 1], f32, isOutput=False)
    outd = nc.declare_dram_parameter("out", [1, R], f32, isOutput=True)

    Relu = mybir.ActivationFunctionType.Relu
    Sigmoid = mybir.ActivationFunctionType.Sigmoid
    Identity = mybir.ActivationFunctionType.Identity
    Add = mybir.AluOpType.add
    Mult = mybir.AluOpType.mult

    with tile.TileContext(nc) as tc:
        with (
            tc.tile_pool(name="weights", bufs=1) as wp,
            tc.tile_pool(name="summary", bufs=1) as sp,
            tc.tile_pool(name="acts", bufs=16) as ap,
            tc.tile_pool(name="tow", bufs=4) as twp,
            tc.tile_pool(name="small", bufs=1) as smp,
            tc.tile_pool(name="zwork", bufs=2) as zw,
            tc.tile_pool(name="psum", bufs=7, space="PSUM") as pp,
            tc.tile_pool(name="zpsum", bufs=1, space="PSUM") as zp,
        ):
            # --- small/early tiles on the gpsimd SW queue; step 0 only
            # needs the tower weights + biases to start.  The step-0 tower
            # weights + tower data go first on the two HW DGE queues, and
            # the weights are spread over four queues in deadline order
            # (step 0: M2, M3, O2; step 1 adds M1s, O1s on vector/tensor
    # queues so they load in parallel with the step-0 weights). ---
            ball = smp.tile([128, 56], f32, tag="ball", name="ball")
            nc.gpsimd.dma_start(out=ball, in_=balld[:])
            ob3 = smp.tile([1, 1], f32, tag="ob3", name="ob3")
            nc.gpsimd.dma_start(out=ob3, in_=ob3d[:])
            w3c = smp.tile([128, FT], f32, tag="w3c", name="w3c")
            nc.gpsimd.dma_start(out=w3c, in_=w3cd[:])
            # tower weights as DoubleRow pairs, duplicated across all four
            # 32-row PE row groups (M r0 / O r0 / M r1 / O r1) so the four
            # per-m tower matmuls all stream concurrently
            w1t5 = wp.tile([128, 2, NH], f8, tag="w1t5", name="w1t5")
            tow0 = twp.tile([128, 2, R], f8, tag="tow", name="tow")

            _q = [0]
            _engs = [nc.sync, nc.scalar, nc.gpsimd]

            def load_dr_tiles(dram, name):
                tiles = []
                for j in range(KT):
                    t = wp.tile([128, 2, NH], f8, tag=f"{name}{j}",
                                name=f"{name}{j}")
                    _engs[_q[0] % 3].dma_start(out=t, in_=dram[j])
                    _q[0] += 1
                    tiles.append(t)
                return tiles

            # k0-deadline order: the row-group-0/1 tower quarters gate the
            # very first matmuls; mw2 (k0's M2, the next deadline at ~1MB)
            # jumps ahead of the row-group-2/3 quarters, which are only
            # needed once l1_k0 reaches its r=1 waves.
            nc.sync.dma_start(out=w1t5[0:32, :, :], in_=mw1td[:])
            nc.scalar.dma_start(out=w1t5[32:64, :, :], in_=ow1td[:])
            nc.sync.dma_start(out=tow0[0:32, :, :], in_=towd[0])
            nc.scalar.dma_start(out=tow0[32:64, :, :], in_=towd[0])
            mw2 = load_dr_tiles(mw2d, "mw2")  # sync, scalar, gpsimd, sync
            nc.scalar.dma_start(out=w1t5[96:128, :, :], in_=ow1td[:])
            nc.gpsimd.dma_start(out=w1t5[64:96, :, :], in_=mw1td[:])
            nc.gpsimd.dma_start(out=tow0[64:96, :, :], in_=towd[0])
            nc.scalar.dma_start(out=tow0[96:128, :, :], in_=towd[0])
            _q[0] = 1  # resume round-robin on scalar
            mw3 = load_dr_tiles(mw3d, "mw3")
            ow2 = load_dr_tiles(ow2d, "ow2")
            mw1s = load_dr_tiles(mw1sd, "mw1s")
            ow1s = load_dr_tiles(ow1sd, "ow1s")

            ones = smp.tile([128, 1], bf, tag="ones", name="ones")
            nc.vector.memset(ones, 1.0)
            w3b = smp.tile([128, FT], bf, tag="w3b", name="w3b")
            nc.vector.tensor_copy(w3b[:], w3c[:])

            # --- summary double buffer (fp8 pair-tiles).  sA is never
            # read at k=0 (step-0 layer 1 is tower-only), so no
            # initialization is needed. ---
            sA = [[sp.tile([128, 2, RB], f8, tag=f"sA{j}_{r}",
                           name=f"sA{j}_{r}") for r in range(NR)]
                  for j in range(KT)]
            sB = [[sp.tile([128, 2, RB], f8, tag=f"sB{j}_{r}",
                           name=f"sB{j}_{r}") for r in range(NR)]
                  for j in range(KT)]

            # --- product accumulators ---
            pacc = []
            for r in range(NR):
                t = smp.tile([1, RB], f32, tag=f"pacc{r}", name=f"pacc{r}")
                nc.vector.memset(t, 1.0)
                pacc.append(t)

            # bias column: layer l in {0:Mb1 1:Mb2 2:Mb3 3:Ob1 4:Ob2} at
            # l*8+m; step-0 fused biases (b + agg@W1s) at 40+m (M), 48+m (O)
            def relu_epilogue(ot, ps, bias_col, m):
                """Bias+relu out of PSUM; alternate ACT/DVE by m so neither
                engine head-of-line-blocks the PE's psum bank rotation."""
                bias = ball[:, ds(bias_col, 1)]
                if m % 2 == 0:
                    nc.scalar.activation(ot, ps[:], Relu, bias=bias)
                else:
                    nc.vector.tensor_scalar(ot, ps[:], bias, 0.0, Add,
                                            mybir.AluOpType.max)

            def pair_tiles(tag):
                return [[ap.tile([128, 2, RB], f8, tag=tag, name=tag)
                         for _ in range(NR)] for _ in range(KT)]

            def l1_fused(scur, tow_t, include_m=True, deferred=()):
                """Fused M/O layer 1.  Per output tile m: all DoubleRow
                summary matmuls (stationary reused across the NR row
                blocks), then the contraction-64 tower closers with M on
                PE rows 0-63 and O on rows 64-127, issued adjacently so
                each M/O pair streams concurrently.  `deferred` is a list
                of closures (the previous step's DVE head-chain ops),
                drained ~4 per m-iteration so they interleave with this
                layer's epilogues in the DVE FIFO instead of forming a
                9us burst that blocks PSUM bank release."""
                m1p = pair_tiles("m1p") if include_m else None
                h1p = pair_tiles("h1p")
                dq = list(deferred)
                for m in range(FT):
                    psm = [pp.tile([128, RB], f32, tag="ps", name="psm")
                           for _ in range(NR)] if include_m else None
                    pso = [pp.tile([128, RB], f32, tag="ps", name="pso")
                           for _ in range(NR)]
                    for j in range(KT):
                        if include_m:
                            for r in range(NR):
                                nc.tensor.matmul(
                                    psm[r][:], mw1s[j][:, :, ts(m, 128)],
                                    scur[j][r][:], start=(j == 0),
                                    stop=False, perf_mode=DR)
                        for r in range(NR):
                            nc.tensor.matmul(
                                pso[r][:], ow1s[j][:, :, ts(m, 128)],
                                scur[j][r][:], start=(j == 0),
                                stop=False, perf_mode=DR)
                    for r in range(NR):
                        mb, ob = 64 * r, 64 * r + 32
                        if include_m:
                            nc.tensor.matmul(
                                psm[r][:], w1t5[mb:mb + 32, :, ts(m, 128)],
                                tow_t[mb:mb + 32, :, ts(r, RB)],
                                start=False, stop=True, perf_mode=DR,
                                tile_position=(mb, 0))
                        nc.tensor.matmul(
                            pso[r][:], w1t5[ob:ob + 32, :, ts(m, 128)],
                            tow_t[ob:ob + 32, :, ts(r, RB)],
                            start=False, stop=True, perf_mode=DR,
                            tile_position=(ob, 0))
                    q, i = divmod(m, 2)
                    # M epilogues first: the next layer's j=3 matmuls wait
                    # on the last m1 pair, while h1 isn't read until O2
                    if include_m:
                        for r in range(NR):
                            relu_epilogue(m1p[q][r][:, i, :], psm[r], m, m)
                    for r in range(NR):
                        relu_epilogue(h1p[q][r][:, i, :], pso[r], 24 + m,
                                      m + 1)
                    # hold the drain until m=2: at layer start the DVE is
                    # still clearing the previous step's O2 epilogues, and
                    # extra queue depth there delays PSUM bank release
                    take = 0 if m < 2 else (len(dq) + FT - 1 - m) // (FT - m)
                    for _ in range(take):
                        dq.pop(0)()
                return m1p, h1p

            def l1_k0(tow_t):
                """Step 0: summary is the broadcast aggregate, folded into
                the bias on the host, so layer 1 is just the tower matmul."""
                m1p = pair_tiles("m1p")
                h1p = pair_tiles("h1p")
                for m in range(FT):
                    psm = [pp.tile([128, RB], f32, tag="ps", name="psm")
                           for _ in range(NR)]
                    pso = [pp.tile([128, RB], f32, tag="ps", name="pso")
                           for _ in range(NR)]
                    for r in range(NR):
                        mb, ob = 64 * r, 64 * r + 32
                        nc.tensor.matmul(
                            psm[r][:], w1t5[mb:mb + 32, :, ts(m, 128)],
                            tow_t[mb:mb + 32, :, ts(r, RB)],
                            start=True, stop=True, perf_mode=DR,
                            tile_position=(mb, 0))
                        nc.tensor.matmul(
                            pso[r][:], w1t5[ob:ob + 32, :, ts(m, 128)],
                            tow_t[ob:ob + 32, :, ts(r, RB)],
                            start=True, stop=True, perf_mode=DR,
                            tile_position=(ob, 0))
                    q, i = divmod(m, 2)
                    for r in range(NR):
                        relu_epilogue(m1p[q][r][:, i, :], psm[r], 40 + m, m)
                        relu_epilogue(h1p[q][r][:, i, :], pso[r], 48 + m,
                                      m + 1)
                return m1p, h1p

            def layer_dr(rhs, ws, writer):
                """1024x1024 DoubleRow layer: per output tile m, KT
                contraction matmuls x NR row blocks, stationary reused
                across row blocks."""
                for m in range(FT):
                    pss = [pp.tile([128, RB], f32, tag="ps", name="ps")
                           for _ in range(NR)]
                    for j in range(KT):
                        for r in range(NR):
                            nc.tensor.matmul(
                                pss[r][:], ws[j][:, :, ts(m, 128)],
                                rhs[j][r][:], start=(j == 0),
                                stop=(j == KT - 1), perf_mode=DR)
                    for r in range(NR):
                        writer(m, r, pss[r])

            def flush_zjobs(zjobs):
                # both row blocks' [1, RB] reduces share one PSUM bank at
                # partitions 0 / 32 (tile_position col groups), freeing a
                # bank for the main psum pool.  (DoubleRow variants of this
                # reduce don't work: small/offset dst partition counts fail
                # walrus's s3d3_mm_valid_dst_partition check, and an fp8
                # hi/lo split of g via mixed-dtype DVE tensor_tensor gave
                # 5.5e-2 rel err on HW — sim/HW divergence.)
                if not zjobs:
                    return
                zt = zp.tile([33, RB], f32, tag="z", name="zps")
                for gb, r in zjobs:
                    zv = zt[ds(32 * r, 1), :]
                    nc.tensor.matmul(zv, ones[:], gb[:],
                                     start=True, stop=True,
                                     tile_position=(0, 32 * r))
                    pr = smp.tile([1, RB], f32, tag=f"pr{r}", name=f"pr{r}")
                    nc.scalar.activation(pr[:], zv, Sigmoid, bias=ob3[:])
                    nc.vector.tensor_mul(pacc[r][:], pacc[r][:], pr[:])

            def head_chain_ops(h2, r, zjobs):
                """The k<K-1 output head as a list of closures: mult/add
                chain on the DVE, drained interleaved into the next step's
                layer-1 loop."""
                st = {}
                ops = []

                def op0():
                    g = zw.tile([128, RB], f32, tag="g", name="g")
                    nc.vector.tensor_scalar(
                        g[:], h2[0][r][:], w3c[:, ds(0, 1)], None, Mult)
                    st["g"] = g
                ops.append(op0)
                for i in range(1, FT):
                    def opm(i=i):
                        t = zw.tile([128, RB], f32, tag="t", name="t",
                                    bufs=3)
                        nc.vector.tensor_scalar(
                            t[:], h2[i][r][:], w3c[:, ds(i, 1)], None, Mult)
                        st["t"] = t

                    def opa():
                        nc.vector.tensor_tensor(st["g"][:], st["g"][:],
                                                st["t"][:], Add)
                    ops += [opm, opa]

                def opc():
                    gb = zw.tile([128, RB], bf, tag="gb", name="gb", bufs=4)
                    nc.vector.tensor_copy(gb[:], st["g"][:])
                    zjobs.append((gb, r))
                ops.append(opc)
                return ops

            def o2_final(h1):
                """Final-step O2 + output head.  The head g = sum_i
                h2_i * w3_i runs as 8 accumulating [128,1]-stationary bf16
                matmuls per row block on the (otherwise idle) PE, emitted
                after both row blocks' DoubleRow streams, so the
                post-matmul tail is one epilogue + ~3.5us of head matmuls
                instead of a ~12us DVE mult/add-tree chain."""
                h2o = [[None] * FT for _ in range(NR)]
                for r in range(NR):
                    for m in range(FT):
                        ps = pp.tile([128, RB], f32, tag="ps", name="ps")
                        for j in range(KT):
                            nc.tensor.matmul(
                                ps[:], ow2[j][:, :, ts(m, 128)],
                                h1[j][r][:], start=(j == 0),
                                stop=(j == KT - 1), perf_mode=DR)
                        h2t = ap.tile([128, RB], bf, tag="h2", name="h2")
                        relu_epilogue(h2t[:], ps, 32 + m, m)
                        h2o[r][m] = h2t
                zt = zp.tile([33, RB], f32, tag="z", name="zfin",
                             uniquify=False)
                for r in range(NR):
                    zv = zt[ds(32 * r, 1), :]
                    for m in range(FT):
                        nc.tensor.matmul(
                            zv, w3b[:, ds(m, 1)], h2o[r][m][:],
                            start=(m == 0), stop=(m == FT - 1),
                            tile_position=(0, 32 * r))
                    pr = smp.tile([1, RB], f32, tag=f"pr{r}", name=f"pr{r}")
                    nc.scalar.activation(pr[:], zv, Sigmoid, bias=ob3[:])
                    nc.vector.tensor_mul(pacc[r][:], pacc[r][:], pr[:])

            scur, snxt = sA, sB
            zjobs = []
            pending = []
            for k in range(K):
                if k == 0:
                    tow_t = tow0
                else:
                    tow_t = twp.tile([128, 2, R], f8, tag="tow", name="tow")
                    for b in range(4):
                        nc.gpsimd.dma_start(
                            out=tow_t[32 * b:32 * b + 32, :, :],
                            in_=towd[k])

                if k == 0:
                    m1, h1 = l1_k0(tow_t)
                elif k == K - 1:
                    # the final scan carry is discarded by the reference, so
                    # the last step's M branch (M1/M2/M3) is dead code
                    _, h1 = l1_fused(scur, tow_t, include_m=False,
                                     deferred=pending)
                    pending = []
                    m1 = None
                else:
                    m1, h1 = l1_fused(scur, tow_t, deferred=pending)
                    pending = []

                if k < K - 1:
                    m2p = pair_tiles("m2p")

                    def w_m2(m, r, ps):
                        q, i = divmod(m, 2)
                        relu_epilogue(m2p[q][r][:, i, :], ps, 8 + m, m)

                    layer_dr(m1, mw2, w_m2)
                    # previous step's output head (its DVE reduce is long
                    # done, so the sigmoid never head-of-line-blocks the
                    # ACT queue)
                    flush_zjobs(zjobs)
                    zjobs = []

                    def w_m3(m, r, ps):
                        q, i = divmod(m, 2)
                        relu_epilogue(snxt[q][r][:, i, :], ps, 16 + m, m)

                    layer_dr(m2p, mw3, w_m3)
                else:
                    flush_zjobs(zjobs)
                    zjobs = []

                if k < K - 1:
                    h2 = [[None] * NR for _ in range(FT)]

                    def w_o2(m, r, ps):
                        t = ap.tile([128, RB], bf, tag="h2", name="h2")
                        relu_epilogue(t[:], ps, 32 + m, m)
                        h2[m][r] = t

                    layer_dr(h1, ow2, w_o2)
                    # g = sum_i h2_i * w3_i on the DVE (per-partition
                    # scalars), reduced across partitions next step by a
                    # ones-matmul.  Emission is deferred into the next
                    # step's layer-1 loop (chains for r=0/r=1 interleaved).
                    c0 = head_chain_ops(h2, 0, zjobs)
                    c1 = head_chain_ops(h2, 1, zjobs)
                    pending = [op for pair in zip(c0, c1) for op in pair]
                else:
                    o2_final(h1)

                scur, snxt = snxt, scur

            for r in range(NR):
                nc.sync.dma_start(out=outd[:, ts(r, RB)], in_=pacc[r][:])

    nc.finalize()
    return nc


def _get_nc():
    global _BUILT
    if _BUILT is None:
        _BUILT = _build()
    return _BUILT


def _dr_quant(W):
    """[NH, NH] f32 -> [KT, 128, 2, NH] e4m3 DoubleRow interleave:
    out[j, p, i, m] = W[256j + 128i + p, m]."""
    return np.ascontiguousarray(
        W.reshape(KT, 2, 128, NH).transpose(0, 2, 1, 3)).astype(F8)


def _dr_pair32(W):
    """[64, X] f32 -> [32, 2, X] e4m3: out[p, i, x] = W[32i + p, x]."""
    return np.ascontiguousarray(
        W.reshape(2, 32, -1).transpose(1, 0, 2)).astype(F8)


def _prep_inputs(inputs):
    f32 = np.float32
    towers = np.asarray(inputs["towers"], dtype=f32)
    agg = np.asarray(inputs["aggregate"], dtype=f32)
    MW1 = np.asarray(inputs["MW1"], dtype=f32)
    OW1 = np.asarray(inputs["OW1"], dtype=f32)

    def col8(v):
        return np.asarray(v, f32).reshape(FT, 128).T

    # step-0 biases with the broadcast-aggregate layer-1 contribution folded
    b0m = np.asarray(inputs["Mb1"], f32) + agg[0] @ MW1[:NH]
    b0o = np.asarray(inputs["Ob1"], f32) + agg[0] @ OW1[:NH]

    shared = {
        "mw1s": _dr_quant(MW1[:NH]),
        "mw1t": _dr_pair32(MW1[NH:]),
        "mw2": _dr_quant(np.asarray(inputs["MW2"], f32)),
        "mw3": _dr_quant(np.asarray(inputs["MW3"], f32)),
        "ow1s": _dr_quant(OW1[:NH]),
        "ow1t": _dr_pair32(OW1[NH:]),
        "ow2": _dr_quant(np.asarray(inputs["OW2"], f32)),
        "w3c": np.ascontiguousarray(col8(np.asarray(inputs["OW3"], f32))),
        "ball": np.ascontiguousarray(np.concatenate(
            [col8(inputs[b]) for b in ("Mb1", "Mb2", "Mb3", "Ob1", "Ob2")]
            + [col8(b0m), col8(b0o)], axis=1)),
        "ob3": np.asarray(inputs["Ob3"], f32).reshape(1, 1),
    }
    in_maps = []
    for c in range(N_CORES):
        tc_ = towers[c * R:(c + 1) * R]          # (R, K, NI)
        t = tc_.transpose(1, 2, 0)               # (K, NI, R)
        towT = np.ascontiguousarray(
            t.reshape(K, 2, NI // 2, R).transpose(0, 2, 1, 3)).astype(F8)
        in_maps.append({"tow": towT, **shared})
    return in_maps


def _run(inputs, trace=False):
    nc = _get_nc()
    in_maps = _prep_inputs(inputs)
    res = run_bass_kernel_spmd(nc, in_maps, list(range(N_CORES)), trace=trace)
    out = np.concatenate([res.results[c]["out"][0] for c in range(N_CORES)])
    return out.astype(np.float32), res


def kernel(**inputs):
    out, _ = _run(inputs, trace=False)
    return out
